# revision 85
# baseline (speedup 1.0000x reference)
"""TRN2 Bass/Tile kernel: BatchNorm1d + 4-head self-attention + out-projection.

Reference computation (b=4, c=256, n=4096, heads=4, d=64):
    xn   = BN(x)  (training-mode stats over batch+length)
    qkv  = w_qkv @ xn ;  q,k,v  (q scaled by d^-0.5)
    out  = softmax(q^T k) @ v^T  per (batch, head)
    y    = w_out @ out + b_out

Sharding over 8 NeuronCores: core i handles (batch i//2, query-half i%2).
Keys/values are processed in the core-local order [mine, other] (softmax and
attention are invariant to key permutation).

Design notes (see git-less history in the per-session transcript):
  - NO cross-core collective: every core also receives the other 3 batches
    (bf16->fp8 for stats only) and computes the EXACT global BN statistics
    locally (own batch via DVE bn_stats, part of the rest via ACT
    Copy/Square accum_out sums).  This makes per-core runtime immune to
    core launch skew, which dominated the AllReduce variant (up to 130us
    of wait on the slowest core).
  - BN scale is folded into the QKV weights (w_qkvT rows *= s) instead of
    rescaling x; the BN shift becomes per-output-channel biases (W @ shift,
    3 tiny PE matmuls) fused into the PSUM->SBUF copies.  The k-projection
    bias is dropped entirely: it shifts all scores of a query equally and
    softmax cancels it.
  - x / weights / attn / q / k are all bf16 (same 1 cycle/row PE rate as
    float32r, half the DMA/SBUF traffic and faster PE weight loads).
  - Attention in the transposed-score layout S^T[key, query], G=2 key
    chunks per exp group, PSUM: 3 sp buffers (2-group lookahead) + 2 mm.
  - exp split across engines: most groups on ACT (table exp), 4 of 16 on
    the DVE as a Schraudolph bf16 bit-trick (one fused mult+add to int16,
    bitcast to bf16; ~3% max rel err on those groups only).
  - AV: lhsT = vT-block [128key, 65] bf16 (64 v channels + ones column ->
    softmax denominator for free); no zero padding (PSUM rows 65:127 are
    never read).  Normalization: denominator staged to SBUF,
    reciprocal_approx_fast, gpsimd partition_broadcast, fused multiply;
    deferred into the middle of the NEXT block so it never blocks the
    DVE exp groups.
  - ~170 tiny keep-warm PE matmuls cover the startup phase so the HAM
    activity monitor does not down-clock the engines before attention.
"""

import numpy as np

import concourse.bacc as bacc
import concourse.tile as tile
from concourse import mybir
from concourse.bass_utils import run_bass_kernel_spmd

B, C, N = 4, 256, 4096
H, D = 4, 64
P = 128
CT = C // P            # 2 channel tiles of 128
RB = 2                 # row blocks for q/k rows (256 = 2*128)
NH = N // 2            # 2048 queries per core
QS = 512               # query subtile (1 PSUM bank of fp32)
NQS = NH // QS         # 4
KC = 128               # key chunk (matmul stationary width)
NKC = N // KC          # 32
G = 2                  # key chunks per exp group (2 PSUM banks)
NG = (NKC + G - 1) // G
EPS = 1e-5
SCALE = D ** -0.5
F32 = mybir.dt.float32
F32R = mybir.dt.float32r
BF16 = mybir.dt.bfloat16
MMDT = F32R  # dtype for q/k tiles feeding the scores matmuls (TF32 on HW)
XDT = BF16   # dtype for x / weights / attn feeding the projection matmuls
F8 = mybir.dt.float8e4
NCORES = 8
USE_RECIP_APPROX = True
DUMMY_WARM = 170   # keep-warm PE matmuls during the startup phase (HAM clock)
# groups per (j,h) block whose exp runs on the DVE as a Schraudolph bf16
# bit-trick instead of on the (bottleneck) ACT engine.  exp(x) ~ bitcast
# int16(x * 128*log2(e) + (127*128 - c)): one fused mult+add per group.
DVE_EXP_GROUPS = (3, 7, 11, 15)
DEFER_AT = 8   # group index in the NEXT block at which the deferred
               # normalize/outproj of the previous block is emitted
SCH_A = 184.6650244    # 2^7 / ln 2
SCH_B = 16250.65       # 127*128 - c_opt (half-way rounding compensation)


def _body(tc, x_mine, x_other, x_rest, w_qkvT, w_outT, bn_w, bn_b, b_out, out):
    from contextlib import ExitStack

    nc = tc.nc
    AF = mybir.ActivationFunctionType
    OP = mybir.AluOpType

    with ExitStack() as ctx:
        big = ctx.enter_context(tc.tile_pool(name="big", bufs=1))
        small = ctx.enter_context(tc.tile_pool(name="small", bufs=1))
        epool = ctx.enter_context(tc.tile_pool(name="epool", bufs=4))
        rpool = ctx.enter_context(tc.tile_pool(name="rpool", bufs=1))
        opool = ctx.enter_context(tc.tile_pool(name="opool", bufs=2))
        spool = ctx.enter_context(tc.tile_pool(name="spool", bufs=3, space="PSUM"))
        mmpool = ctx.enter_context(tc.tile_pool(name="mmpool", bufs=2, space="PSUM"))

        # ---- loads: x_mine first (BN stats critical path) ---------------
        xn_sb = big.tile([P, CT, N], XDT, tag="xnattn")  # RAW x, key order [mine|other]
        xm_r = x_mine.rearrange("(ct p) n -> p ct n", p=P)
        for ct in range(CT):
            for half in range(2):
                nc.sync.dma_start(
                    out=xn_sb[:, ct, half * (NH // 2) : (half + 1) * (NH // 2)],
                    in_=xm_r[:, ct, half * (NH // 2) : (half + 1) * (NH // 2)],
                )
        nc.sync.dma_start(
            out=xn_sb[:, :, NH:N], in_=x_other.rearrange("(ct p) n -> p ct n", p=P)
        )
        wq_sb = big.tile([P, CT, 3 * C], XDT)
        nc.sync.dma_start(
            out=wq_sb, in_=w_qkvT.rearrange("(ct p) o -> p ct o", p=P)
        )
        wo_sb = big.tile([P, CT, C], XDT)
        nc.sync.dma_start(out=wo_sb, in_=w_outT.rearrange("(ct p) o -> p ct o", p=P))
        bnw_sb = small.tile([P, CT, 1], F32)
        nc.sync.dma_start(out=bnw_sb, in_=bn_w)
        bnb_sb = small.tile([P, CT, 1], F32)
        nc.sync.dma_start(out=bnb_sb, in_=bn_b)
        bo_sb = small.tile([P, CT, 1], F32)
        nc.sync.dma_start(out=bo_sb, in_=b_out)

        # ---- PE keep-warm ------------------------------------------------
        # The HAM activity monitor drops engine clocks ~20% when the PE sits
        # idle; the whole startup phase (DMA + BN stats) would otherwise run
        # with a cold PE and the attention phase starts throttled.  A stream
        # of tiny matmuls keeps the PE marked active until real work arrives.
        if DUMMY_WARM:
            dum_sb = small.tile([1, 256], BF16)
            nc.vector.memset(dum_sb, 1.0)
            for _ in range(DUMMY_WARM):
                scrap = mmpool.tile([P, QS], F32, tag="mm", name="scrap")
                nc.tensor.matmul(
                    out=scrap[0:1, 0:256],
                    lhsT=dum_sb[0:1, 0:1],
                    rhs=dum_sb,
                    start=True,
                    stop=True,
                )

        # ---- BN stats: EXACT global stats computed locally --------------
        # Every core receives the 3 other batches as bf16 (x_rest) in
        # addition to its own full batch row (x_mine + x_other = all 4096
        # columns of batch b).  Global (b, n) statistics are then a local
        # bn_stats/bn_aggr over all 4 batches: no cross-core collective,
        # so per-core runtime is immune to core launch skew.
        NRC = 4            # x_rest DMA chunks per batch (1024 fp8 cols)
        RCW = N // NRC
        SG = N // 512      # own-batch 512-col stat groups per ct
        NCH = 3 * NRC
        ACT_CH = (2, 5, 8, 11)   # chunks reduced on ACT via accum_out sums
        NACT = len(ACT_CH)
        RG = RCW // 512
        NDVE_R = (NCH - NACT) * RG
        NREC = SG + NDVE_R         # bn_stats records per ct
        NS = N + NDVE_R * 512      # samples covered by bn_stats records
        NT = B * N                 # total samples per channel
        stat6 = small.tile([P, CT, NREC, 6], F32)
        for ct in range(CT):
            xm = xn_sb[:, ct, :].rearrange("p (s f) -> p s f", f=512)
            for s in range(SG):
                nc.vector.bn_stats(out=stat6[:, ct, s, :], in_=xm[:, s, :])
        stg = ctx.enter_context(tc.tile_pool(name="stg", bufs=4))
        trash = small.tile([P, RCW], BF16)
        acc_x = small.tile([P, CT, NACT], F32)
        acc_x2 = small.tile([P, CT, NACT], F32)
        ci_dve = 0
        ci_act = 0
        for rb_ in range(3):
            for chunk in range(NRC):
                st = stg.tile([P, CT, RCW], F8, tag="stg")
                nc.sync.dma_start(out=st, in_=x_rest[rb_, chunk])
                if rb_ * NRC + chunk in ACT_CH:
                    # ACT computes plain sums: Sum(x) via Copy-accumulate,
                    # Sum(x^2) via Square-accumulate
                    for ct in range(CT):
                        nc.scalar.activation(
                            out=trash, in_=st[:, ct, :], func=AF.Copy,
                            accum_out=acc_x[:, ct, ci_act : ci_act + 1],
                        )
                        nc.scalar.activation(
                            out=trash, in_=st[:, ct, :], func=AF.Square,
                            accum_out=acc_x2[:, ct, ci_act : ci_act + 1],
                        )
                    ci_act += 1
                else:
                    for ct in range(CT):
                        xr = st[:, ct, :].rearrange("p (s f) -> p s f", f=512)
                        for s in range(RG):
                            nc.vector.bn_stats(
                                out=stat6[:, ct, SG + ci_dve * RG + s, :],
                                in_=xr[:, s, :],
                            )
                    ci_dve += 1
        mv = small.tile([P, CT, 2], F32)
        for ct in range(CT):
            nc.vector.bn_aggr(out=mv[:, ct, :], in_=stat6[:, ct])
        # combine: totals = bn_aggr subset (NS samples) + ACT sums
        sum_t = small.tile([P, CT, 1], F32)
        nc.vector.tensor_reduce(
            out=sum_t, in_=acc_x, axis=mybir.AxisListType.X,
            op=mybir.AluOpType.add,
        )
        sq_t = small.tile([P, CT, 1], F32)
        nc.vector.tensor_reduce(
            out=sq_t, in_=acc_x2, axis=mybir.AxisListType.X,
            op=mybir.AluOpType.add,
        )
        msq_s = small.tile([P, CT, 1], F32)
        nc.vector.tensor_mul(out=msq_s, in0=mv[:, :, 0:1], in1=mv[:, :, 0:1])
        e2_s = small.tile([P, CT, 1], F32)
        nc.vector.tensor_add(out=e2_s, in0=mv[:, :, 1:2], in1=msq_s)
        # sum_t += mean_s * NS ; sq_t += e2_s * NS
        tmp_s = small.tile([P, CT, 1], F32)
        nc.vector.tensor_scalar_mul(out=tmp_s, in0=mv[:, :, 0:1], scalar1=float(NS))
        nc.vector.tensor_add(out=sum_t, in0=sum_t, in1=tmp_s)
        nc.vector.tensor_scalar_mul(out=tmp_s, in0=e2_s, scalar1=float(NS))
        nc.vector.tensor_add(out=sq_t, in0=sq_t, in1=tmp_s)
        mvg = small.tile([P, CT, 2], F32)
        nc.vector.tensor_scalar_mul(
            out=mvg[:, :, 0:1], in0=sum_t, scalar1=1.0 / NT
        )
        nc.vector.tensor_scalar_mul(out=tmp_s, in0=sq_t, scalar1=1.0 / NT)
        nc.vector.tensor_mul(
            out=mvg[:, :, 1:2], in0=mvg[:, :, 0:1], in1=mvg[:, :, 0:1]
        )
        nc.vector.tensor_sub(out=mvg[:, :, 1:2], in0=tmp_s, in1=mvg[:, :, 1:2])
        mv = mvg

        # ---- padding fills (run during the AllReduce wait) --------------
        # q/k are stored zero-padded to 128 partitions per head (rows 64:128
        # are zeros): a K=64 matmul streams its operands at HALF the SBUF
        # bandwidth, padding the contraction to 128 restores full rate.
        # vT is padded to 128 columns per head (cols 65:128 zero), kept in
        # bf16 together with exp(S) so the AV matmul gets fast-weight-load.
        # All fills run on the (otherwise idle) gpsimd engine while the DVE
        # crunches BN stats and DMA streams x_rest.
        # vT has 65 columns per head: 64 v-channels + the ones column that
        # produces the softmax denominator.  (No zero padding to 128: the
        # extra stationary columns would only produce PSUM rows 65:127,
        # which nothing reads.)
        q_pad = big.tile([P, H, NH], BF16)
        k_pad = big.tile([P, H, N], BF16)
        vT_pad = big.tile([P, NKC, H, D + 1], BF16)
        for h in range(H):
            nc.gpsimd.memset(k_pad[D:P, h].bitcast(F32), 0.0)
            nc.gpsimd.memset(q_pad[D:P, h].bitcast(F32), 0.0)
        nc.gpsimd.memset(vT_pad[:, :, :, D : D + 1], 1.0)
        eps_sb = small.tile([P, 1], F32)
        nc.vector.memset(eps_sb, EPS)

        # ---- global mean/var -> s = bn_w * rstd, shift = bn_b - mean*s --
        mean_g = mv[:, :, 0:1]
        var_g = mv[:, :, 1:2]
        sd = small.tile([P, CT, 1], F32)
        nc.scalar.activation(out=sd, in_=var_g, func=AF.Sqrt, bias=eps_sb)
        rstd = small.tile([P, CT, 1], F32)
        nc.vector.reciprocal(out=rstd, in_=sd)
        s_sb = small.tile([P, CT, 1], F32)
        nc.vector.tensor_mul(out=s_sb, in0=bnw_sb, in1=rstd)
        shift_sb = small.tile([P, CT, 1], F32)
        nc.vector.tensor_mul(out=shift_sb, in0=mean_g, in1=s_sb)
        nc.vector.tensor_sub(out=shift_sb, in0=bnb_sb, in1=shift_sb)
        shift_bf = small.tile([P, CT, 1], BF16)
        nc.vector.tensor_copy(out=shift_bf, in_=shift_sb)

        # ---- qkv bias = W @ shift (with ORIGINAL weights), then fold ----
        # diag(s) into the weights in place.  q/k biases come out as
        # per-partition columns [128,1]; the v bias as a [1,256] row that is
        # partition-broadcast for the (free-axis) fused add on vT copies.
        bias_ps = mmpool.tile([P, QS], F32, tag="mm")
        for rb in range(2 * RB):  # q rb0, q rb1, k rb0, k rb1
            for ct in range(CT):
                nc.tensor.matmul(
                    out=bias_ps[:, rb : rb + 1],
                    lhsT=wq_sb[:, ct, rb * P : (rb + 1) * P],
                    rhs=shift_bf[:, ct],
                    start=(ct == 0),
                    stop=(ct == CT - 1),
                )
        vb_ps = mmpool.tile([P, QS], F32, tag="mm")
        for ct in range(CT):
            nc.tensor.matmul(
                out=vb_ps[0:1, 0:C],
                lhsT=shift_bf[:, ct],
                rhs=wq_sb[:, ct, 2 * C : 3 * C],
                start=(ct == 0),
                stop=(ct == CT - 1),
            )
        qkb_sb = small.tile([P, 2 * RB], F32)
        nc.vector.tensor_copy(out=qkb_sb, in_=bias_ps[:, 0 : 2 * RB])
        vb_row = small.tile([1, C], F32)
        nc.vector.tensor_copy(out=vb_row, in_=vb_ps[0:1, 0:C])
        vbias_bc = small.tile([P, H, D], F32)
        nc.gpsimd.partition_broadcast(
            vbias_bc.rearrange("p h d -> p (h d)"), vb_row
        )

        # fold diag(s) into the weights, q columns first so q matmuls can
        # start while k/v columns are still being scaled (d^-0.5 is folded
        # into the q columns host-side)
        for sec in range(3):
            for ct in range(CT):
                nc.vector.tensor_scalar_mul(
                    out=wq_sb[:, ct, sec * C : (sec + 1) * C],
                    in0=wq_sb[:, ct, sec * C : (sec + 1) * C],
                    scalar1=s_sb[:, ct],
                )

        # ---- QKV projections (consume RAW x; bias fused into copies) ----
        # PSUM rows 0:64 copy on DVE (tensor_scalar add), rows 64:128 on ACT
        # (activation Identity with per-partition bias) so the copy work is
        # split across both engines.
        def qk_copy(dst, ps, col):
            nc.vector.tensor_scalar_add(
                out=dst[0], in0=ps[0:D, :],
                scalar1=qkb_sb[0:D, col : col + 1],
            )
            nc.scalar.activation(
                out=dst[1], in_=ps[D:P, :], func=AF.Identity,
                bias=qkb_sb[D:P, col : col + 1],
            )

        for j in range(NQS):  # q (my query half only), j0 first
            for rb in range(RB):
                ps = mmpool.tile([P, QS], F32, tag="mm")
                for ct in range(CT):
                    nc.tensor.matmul(
                        out=ps,
                        lhsT=wq_sb[:, ct, rb * P : (rb + 1) * P],
                        rhs=xn_sb[:, ct, j * QS : (j + 1) * QS],
                        start=(ct == 0),
                        stop=(ct == CT - 1),
                    )
                qk_copy(
                    (q_pad[0:D, 2 * rb, j * QS : (j + 1) * QS],
                     q_pad[0:D, 2 * rb + 1, j * QS : (j + 1) * QS]),
                    ps, rb,
                )
        for rb in range(RB):  # k (full length), heads 0/1 (rb=0) first
            for j in range(N // QS):
                ps = mmpool.tile([P, QS], F32, tag="mm")
                for ct in range(CT):
                    nc.tensor.matmul(
                        out=ps,
                        lhsT=wq_sb[:, ct, C + rb * P : C + (rb + 1) * P],
                        rhs=xn_sb[:, ct, j * QS : (j + 1) * QS],
                        start=(ct == 0),
                        stop=(ct == CT - 1),
                    )
                qk_copy(
                    (k_pad[0:D, 2 * rb, j * QS : (j + 1) * QS],
                     k_pad[0:D, 2 * rb + 1, j * QS : (j + 1) * QS]),
                    ps, RB + rb,
                )
        for nb in range(NKC):  # v, produced transposed: [key, (head, d)]
            ps = mmpool.tile([P, C], F32, tag="mm")
            for ct in range(CT):
                nc.tensor.matmul(
                    out=ps,
                    lhsT=xn_sb[:, ct, nb * KC : (nb + 1) * KC],
                    rhs=wq_sb[:, ct, 2 * C : 3 * C],
                    start=(ct == 0),
                    stop=(ct == CT - 1),
                )
            nc.vector.tensor_tensor(
                out=vT_pad[:, nb, :, 0:D],
                in0=ps.rearrange("p (h d) -> p h d", d=D),
                in1=vbias_bc,
                op=mybir.AluOpType.add,
            )

        # ---- attention --------------------------------------------------
        attn_sb = big.tile([P, CT, NH], XDT, tag="xnattn")
        out_r = out.rearrange("(rb p) n -> p rb n", p=P)

        def normalize(avp_, h_, j_):
            # rows 0:D divided by the softmax denominator in row D
            r_sb = rpool.tile([1, QS], F32, tag="r")
            if USE_RECIP_APPROX:
                # reciprocal_approx_fast silently corrupts on PSUM input:
                # stage the denominator row through SBUF first.
                den_sb = rpool.tile([1, QS], F32, tag="den")
                nc.vector.tensor_copy(out=den_sb, in_=avp_[D : D + 1, :])
                nc.vector.reciprocal_approx_fast(out=r_sb, in_=den_sb)
            else:
                nc.vector.reciprocal(out=r_sb, in_=avp_[D : D + 1, :])
            rbc = rpool.tile([D, QS], F32, tag="rbc")
            nc.gpsimd.partition_broadcast(rbc, r_sb)
            nc.vector.tensor_tensor(
                out=attn_sb[(h_ % 2) * D : (h_ % 2) * D + D, h_ // 2,
                            j_ * QS : (j_ + 1) * QS],
                in0=avp_[0:D, :],
                in1=rbc,
                op=mybir.AluOpType.mult,
            )

        def outproj(j_):
            # output projection + bias for this query block (all heads done)
            for rb in range(RB):
                ps = mmpool.tile([P, QS], F32, tag="mm")
                for ct in range(CT):
                    nc.tensor.matmul(
                        out=ps,
                        lhsT=wo_sb[:, ct, rb * P : (rb + 1) * P],
                        rhs=attn_sb[:, ct, j_ * QS : (j_ + 1) * QS],
                        start=(ct == 0),
                        stop=(ct == CT - 1),
                    )
                o_t = opool.tile([P, QS], F32, tag="o")
                nc.vector.tensor_scalar_add(out=o_t, in0=ps, scalar1=bo_sb[:, rb])
                nc.sync.dma_start(
                    out=out_r[:, rb, j_ * QS : (j_ + 1) * QS], in_=o_t
                )

        # normalize/outproj of block X is deferred into the middle of block
        # X+1 so the DVE's Schraudolph groups of X+1 are never queued behind
        # the normalize chain (mmpool bufs=2 keeps avp(X) alive long enough)
        defer = [None]

        def flush_norm():
            if defer[0] is not None:
                avp_, h_, j_ = defer[0]
                defer[0] = None
                normalize(avp_, h_, j_)
                if h_ == H - 1:
                    outproj(j_)

        def emit_av(p):
            # the AV matmuls run one exp-group behind the scores so exp(g-1)
            # always completes while the PE streams S(g): no per-group stall
            e_, g_, gs_, avp_, h_, j_ = p
            for u in range(gs_):
                kc = g_ * G + u
                nc.tensor.matmul(
                    out=avp_[0 : D + 1, :],
                    lhsT=vT_pad[:, kc, h_, :],
                    rhs=e_[:, u, :],
                    start=(kc == 0),
                    stop=(kc == NKC - 1),
                )
            if g_ == NG - 1:
                defer[0] = (avp_, h_, j_)

        pend = None
        for j in range(NQS):
            for h in range(H):
                avp = mmpool.tile([P, QS], F32, tag="mm")
                for g in range(NG):
                    gs = min(G, NKC - g * G)
                    sp = spool.tile([P, G, QS], F32, tag="sp")
                    for u in range(gs):
                        kc = g * G + u
                        nc.tensor.matmul(
                            out=sp[:, u, :],
                            lhsT=k_pad[:, h, kc * KC : (kc + 1) * KC],
                            rhs=q_pad[:, h, j * QS : (j + 1) * QS],
                            start=True,
                            stop=True,
                        )
                    e_sb = epool.tile([P, G, QS], BF16, tag="e")
                    if g in DVE_EXP_GROUPS:
                        nc.vector.tensor_scalar(
                            out=e_sb[:, 0:gs, :].bitcast(mybir.dt.int16),
                            in0=sp[:, 0:gs, :],
                            scalar1=SCH_A,
                            scalar2=SCH_B,
                            op0=mybir.AluOpType.mult,
                            op1=mybir.AluOpType.add,
                        )
                    else:
                        nc.scalar.activation(
                            out=e_sb[:, 0:gs, :], in_=sp[:, 0:gs, :], func=AF.Exp
                        )
                    if pend is not None:
                        emit_av(pend)
                    if g == DEFER_AT:
                        flush_norm()
                    pend = (e_sb, g, gs, avp, h, j)
        emit_av(pend)
        flush_norm()


def build():
    nc = bacc.Bacc(
        "TRN2", target_bir_lowering=False, debug=False, num_devices=NCORES
    )
    x_mine = nc.dram_tensor("x_mine", [C, NH], XDT, kind="ExternalInput").ap()
    x_other = nc.dram_tensor("x_other", [C, NH], XDT, kind="ExternalInput").ap()
    x_rest = nc.dram_tensor(
        "x_rest", [3, 4, P, CT, N // 4], F8, kind="ExternalInput"
    ).ap()
    w_qkvT = nc.dram_tensor("w_qkvT", [C, 3 * C], XDT, kind="ExternalInput").ap()
    w_outT = nc.dram_tensor("w_outT", [C, C], XDT, kind="ExternalInput").ap()
    bn_w = nc.dram_tensor("bn_w", [P, CT, 1], F32, kind="ExternalInput").ap()
    bn_b = nc.dram_tensor("bn_b", [P, CT, 1], F32, kind="ExternalInput").ap()
    b_out = nc.dram_tensor("b_out", [P, CT, 1], F32, kind="ExternalInput").ap()
    out = nc.dram_tensor("out", [C, NH], F32, kind="ExternalOutput").ap()
    with tile.TileContext(nc) as tc:
        _body(tc, x_mine, x_other, x_rest, w_qkvT, w_outT, bn_w, bn_b, b_out, out)
    nc.compile()
    return nc


_nc_cache = None


def make_in_maps(x, bn_weight, bn_bias, w_qkv, w_out, b_out):
    import ml_dtypes

    x = np.ascontiguousarray(np.asarray(x, dtype=np.float32))
    x_bf = x.astype(ml_dtypes.bfloat16)
    x_f8 = x.astype(ml_dtypes.float8_e4m3fn)
    wqT = np.asarray(w_qkv, dtype=np.float32).T.copy()
    wqT[:, 0:C] *= SCALE  # fold d^-0.5 into the q columns
    wqT = wqT.astype(ml_dtypes.bfloat16)
    woT = np.asarray(w_out, dtype=np.float32).T.astype(ml_dtypes.bfloat16)

    def vec_layout(v):
        v = np.asarray(v, dtype=np.float32)
        return np.ascontiguousarray(v.reshape(CT, P).T.reshape(P, CT, 1))

    bnw = vec_layout(bn_weight)
    bnb = vec_layout(bn_bias)
    bo = vec_layout(b_out)
    in_maps = []
    # x_rest layout [3, nchunk, P, CT, 1024]: contiguous per DMA chunk so the
    # stats-stream DMAs are pure sequential reads (c = ct*P + p)
    xr_all = x_f8.reshape(B, CT, P, 4, N // 4).transpose(0, 3, 2, 1, 4)
    for core in range(NCORES):
        bi, half = divmod(core, 2)
        mine = np.ascontiguousarray(x_bf[bi][:, half * NH : (half + 1) * NH])
        other = np.ascontiguousarray(x_bf[bi][:, (1 - half) * NH : (2 - half) * NH])
        rest = np.ascontiguousarray(xr_all[[b for b in range(B) if b != bi]])
        in_maps.append(
            {
                "x_mine": mine,
                "x_other": other,
                "x_rest": rest,
                "w_qkvT": wqT,
                "w_outT": woT,
                "bn_w": bnw,
                "bn_b": bnb,
                "b_out": bo,
            }
        )
    return in_maps


def assemble(results):
    outp = np.empty((B, C, N), np.float32)
    for core in range(NCORES):
        bi, half = divmod(core, 2)
        outp[bi][:, half * NH : (half + 1) * NH] = results[core]["out"]
    return outp


def kernel(x, bn_weight, bn_bias, w_qkv, w_out, b_out):
    global _nc_cache
    if _nc_cache is None:
        _nc_cache = build()
    in_maps = make_in_maps(x, bn_weight, bn_bias, w_qkv, w_out, b_out)
    res = run_bass_kernel_spmd(_nc_cache, in_maps, list(range(NCORES)))
    return assemble(res.results)


if __name__ == "__main__":
    rng = np.random.default_rng(0)
    x = rng.standard_normal((B, C, N), dtype=np.float32)
    w_qkv = rng.standard_normal((3 * C, C), dtype=np.float32) * C**-0.5
    w_out = rng.standard_normal((C, C), dtype=np.float32) * C**-0.5
    y = kernel(
        x,
        np.ones(C, np.float32),
        np.zeros(C, np.float32),
        w_qkv,
        w_out,
        np.zeros(C, np.float32),
    )
    print(y.shape, np.abs(y).max())
